# revision 1
# baseline (speedup 1.0000x reference)
"""Trainium2 Bass kernel for nn_CombinedAMLModel (dense_mlp, 8 NeuronCores).

Sharding: tensor-parallel over the gene axis (20000 genes -> 2500 per core).

Math: per (tech,gene) 1->4->1 MLP + per-gene tech combinor gives, per gene g
and sample s:
    z[g,s] = sum_k c_k * relu(s_k x_t + b_k) + const        (k = (t,h), 12 terms)
with c_k*relu(u) = sgn(c_k)*relu(|c_k| u). The per-gene const is folded into
Cb0 on the host (z feeds CW0 linearly), so the device computes only the
12 signed relu terms.

Per core, s-half pipelined (h0 sweep -> B(h0) || h1 sweep -> B(h1)):
  Phase A (per gene tile of 125, per s-half of 512):
    - 7 relu terms on ACT (fp16), 5 on DVE (two 4x tensor_scalar ops,
      relu+sign folded into the second op); signs for ACT's non-PE terms
      applied by DVE 4x multiplies.
    - 3 of the ACT terms (raw relu) accumulate in PSUM via host-built
      c-diagonal matmuls (fp16, signed weights).
    - the other 9 signed terms accumulate via a DVE/Pool tensor_tensor tree;
      DVE merges psum + sbuf accumulator -> z fp16.
  Phase B: out1[nt] += CW0^T-block @ z (fp16 matmuls, K=125 per step,
    PSUM accumulation over 20 gene tiles), PSUM->fp16 staging on Pool,
    DMA to DRAM partials, chunked fp16 ReduceScatter overlapped.
  Phase D: distributed 2000->200 (each core owns its ReduceScatter slice),
    fp16 AllReduce of the (200,1024) partial, then 200->20->1 replicated.
"""
import os
import sys

sys.path.insert(0, "/opt/trn_rl_repo")

import ml_dtypes
import numpy as np
from contextlib import ExitStack

import concourse.bass as bass
import concourse.tile as tile
from concourse import bacc, mybir
from concourse.bass_utils import run_bass_kernel_spmd

T, S, G, H = 3, 1024, 20000, 4
NCORES = 8
GL = G // NCORES            # genes per core
PT = 125                    # gene-tile partition size
NGT = GL // PT              # gene tiles per core
NK = T * H                  # local terms per gene
N1, N2, N3 = 2000, 200, 20
PN = 125                    # n-tile partition size for layer-1 output
NNT = N1 // PN              # n tiles
SH = 512                    # s-half size (PSUM bank)

# term assignment: k = t*H + h
GEN_ACT = (0, 1, 2, 3, 4, 5, 6)      # generated on ACT (Relu activation)
GEN_DVE = (7, 8, 9, 10, 11)          # generated on DVE (2x tensor_scalar pair)
PE_LANE = (0, 1, 2, 3, 4, 5, 6, 7)   # terms accumulated via c-diag matmul
DVE_LANE = (8, 9, 10, 11)            # signed terms accumulated via DVE adds
NDIAG = len(PE_LANE)                 # diag blocks per tile

f32 = mybir.dt.float32
f16 = mybir.dt.float16
Add = mybir.AluOpType.add
Mult = mybir.AluOpType.mult
Max = mybir.AluOpType.max

LAST_RUN = {}
_CACHE = {}


def _build_program():
    nc = bacc.Bacc("TRN2", target_bir_lowering=False, debug=False,
                   num_devices=NCORES)
    d = {}

    def inp(name, shape, dt=f32):
        d[name] = nc.dram_tensor(name, list(shape), dt, kind="ExternalInput").ap()

    inp("xT", (NGT, PT, T * S), f16)
    inp("scl2", (PT, NGT * NK))
    inp("bia2", (PT, NGT * NK))
    inp("sgn", (PT, NGT * NK))
    inp("pediag", (NGT, PT, NDIAG * PT), f16)
    inp("cw0t", (NNT, PT, NGT * PN), f16)
    inp("cb0", (PN, 2))
    inp("cw1t", (PN, 2 * N2), f16)
    inp("cb1", (100, 2))
    inp("cw2t", (N2, N3), f16)
    inp("cb2", (N3, 1))
    inp("cwft", (N3, 1), f16)
    inp("cbf", (1, 1))
    out_d = nc.dram_tensor("out", [1, S], f32, kind="ExternalOutput").ap()

    Relu = mybir.ActivationFunctionType.Relu
    Ident = mybir.ActivationFunctionType.Identity
    Abs = mybir.ActivationFunctionType.Abs

    with tile.TileContext(nc) as tc, ExitStack() as ctx:
        const = ctx.enter_context(tc.tile_pool(name="const", bufs=1))
        xpool = ctx.enter_context(tc.tile_pool(name="x", bufs=6))
        x1pool = ctx.enter_context(tc.tile_pool(name="x1", bufs=8))
        dgpool = ctx.enter_context(tc.tile_pool(name="dg", bufs=NGT))
        tpool_a = ctx.enter_context(tc.tile_pool(name="ta", bufs=8))
        tpool_u = ctx.enter_context(tc.tile_pool(name="tu", bufs=6))
        tpool_v = ctx.enter_context(tc.tile_pool(name="tv", bufs=8))
        accp = ctx.enter_context(tc.tile_pool(name="acc", bufs=4))
        zpool = ctx.enter_context(tc.tile_pool(name="z", bufs=2 * NGT))
        wpool = ctx.enter_context(tc.tile_pool(name="w0", bufs=3))
        opool = ctx.enter_context(tc.tile_pool(name="o1", bufs=4))
        tpool = ctx.enter_context(tc.tile_pool(name="tail", bufs=1))
        zps = ctx.enter_context(tc.tile_pool(name="zps", bufs=3, space="PSUM"))
        mmps = ctx.enter_context(tc.tile_pool(name="mmps", bufs=4, space="PSUM"))
        dram = ctx.enter_context(tc.tile_pool(name="dram", bufs=1, space="DRAM"))

        # ---- DRAM scratch ----
        NCH = 2
        partial_c = [[dram.tile([N1 // NCH, SH], f16, tag=f"pc{j}{h}",
                                name=f"partial{j}_{h}") for h in range(2)]
                     for j in range(NCH)]
        rs_c = [[dram.tile([PN, SH], f16, tag=f"rs{j}{h}",
                           name=f"rsout{j}_{h}") for h in range(2)]
                for j in range(NCH)]
        partial2 = dram.tile([N2, S], f16, tag="p2", name="partial2")
        summed2 = dram.tile([N2, S], f16, tag="s2", name="summed2")
        ccwarm_in = dram.tile([1, 128], f32, tag="ccwi")
        ccwarm_out = dram.tile([1, 128], f32, tag="ccwo")

        # tiny warm-up collective (absorbs first-rendezvous cost early)
        ccwarm_sb = const.tile([1, 128], f32)
        nc.gpsimd.memset(ccwarm_sb[:], 0.0)
        nc.gpsimd.dma_start(ccwarm_in[:], ccwarm_sb[:])
        nc.gpsimd.collective_compute(
            "AllReduce", Add,
            replica_groups=[list(range(NCORES))],
            ins=[ccwarm_in.opt()], outs=[ccwarm_out.opt()],
        )

        # ---- x loads: tile gt, half h -> (PT, T, SH) fp16 ----
        x_h0 = {}
        x_h1 = {}

        def load_x(gt, h, pool, eng):
            xt = pool.tile([PT, T, SH], f16, tag="x" if h == 0 else "x1",
                           name=f"x{gt}_{h}")
            src = d["xT"][gt].rearrange("p (t s) -> p t s", t=T)
            eng.dma_start(xt[:], src[:, :, h * SH:(h + 1) * SH])
            return xt

        # first few h0 tiles before anything else (gate the A ramp)
        for gt in range(3):
            x_h0[gt] = load_x(gt, 0, xpool, nc.gpsimd)

        # ---- consts ----
        scl2t = const.tile([PT, NGT * NK], f32)
        nc.scalar.dma_start(scl2t[:], d["scl2"][:])
        bia2t = const.tile([PT, NGT * NK], f32)
        nc.scalar.dma_start(bia2t[:], d["bia2"][:])
        sgnt = const.tile([PT, NGT * NK], f32)
        nc.scalar.dma_start(sgnt[:], d["sgn"][:])
        cb0t = const.tile([PN, 2], f32)
        nc.scalar.dma_start(cb0t[:], d["cb0"][:])
        w1t = const.tile([PN, 2 * N2], f16)
        nc.scalar.dma_start(w1t[:], d["cw1t"][:])
        cb1t = const.tile([100, 2], f32)
        nc.scalar.dma_start(cb1t[:], d["cb1"][:])
        cb2t = const.tile([N3, 1], f32)
        nc.scalar.dma_start(cb2t[:], d["cb2"][:])
        cwftt = const.tile([N3, 1], f16)
        nc.scalar.dma_start(cwftt[:], d["cwft"][:])
        cbft = const.tile([1, 1], f32)
        nc.scalar.dma_start(cbft[:], d["cbf"][:])
        cw2tt = const.tile([100, 2 * N3], f16)
        for mc in range(2):
            nc.scalar.dma_start(cw2tt[:, mc * N3:(mc + 1) * N3],
                                d["cw2t"][mc * 100:(mc + 1) * 100, :])

        # per-tile diagonal blocks (3 sgn + 3 alpha), live for both halves
        dg_tiles = {}
        for gt in range(4):
            dg = dgpool.tile([PT, NDIAG * PT], f16, tag="dg", name=f"dg{gt}")
            nc.gpsimd.dma_start(dg[:], d["pediag"][gt])
            dg_tiles[gt] = dg

        # cw0 prefetch: first blocks load during phase A
        HW0 = NGT * PN // 2

        def load_w0(nt):
            w = wpool.tile([PT, NGT * PN], f16, tag="w0", name=f"w0_{nt}")
            nc.gpsimd.dma_start(w[:], d["cw0t"][nt])
            return w

        w_tiles = {nt: load_w0(nt) for nt in range(2)}

        # ---------------- Phase A: one gene tile, one s-half ----------------
        z_half = [[None] * NGT, [None] * NGT]

        def phase_a(gt, h):
            xh = (x_h0 if h == 0 else x_h1)[gt]
            dg = dg_tiles[gt]
            col = lambda k: slice(gt * NK + k, gt * NK + k + 1)
            xs = lambda k: xh[:, k // H, :]

            # ACT terms (PE-lane, raw relu): relu(s*x + b) in fp16
            terms = {}
            for k in GEN_ACT:
                a = tpool_a.tile([PT, SH], f16, tag="ta", name=f"a{gt}_{h}_{k}")
                nc.scalar.activation(a[:], xs(k), Relu,
                                     bias=bia2t[:, col(k)],
                                     scale=scl2t[:, col(k)])
                terms[k] = a

            # DVE terms: u = scl2*x + bia2 ; then relu (PE-lane) or
            # relu*sgn (DVE-lane) -- both 2x fp16 tensor_scalar
            for k in GEN_DVE:
                u = tpool_u.tile([PT, SH], f16, tag="tu", name=f"u{gt}_{h}_{k}")
                nc.vector.tensor_scalar(u[:], xs(k), scl2t[:, col(k)],
                                        bia2t[:, col(k)], Mult, Add)
                v = tpool_v.tile([PT, SH], f16, tag="tv", name=f"v{gt}_{h}_{k}")
                if k in PE_LANE:
                    nc.vector.tensor_scalar(v[:], u[:], 0.0, None, Max)
                else:
                    nc.vector.tensor_scalar(v[:], u[:], 0.0, sgnt[:, col(k)],
                                            Max, Mult)
                terms[k] = v

            # PSUM: 8 c-diag weighted term accumulations
            ps = zps.tile([PT, SH], f32, tag="zps", name=f"zps{gt}_{h}")
            for j, k in enumerate(PE_LANE):
                nc.tensor.matmul(ps[:], dg[:, j * PT:(j + 1) * PT],
                                 terms[k][:], start=(j == 0),
                                 stop=(j == len(PE_LANE) - 1))

            # DVE-lane: 3 adds + psum merge -> z fp16
            items = [terms[k] for k in DVE_LANE]
            d1 = accp.tile([PT, SH], f16, tag="acc", name=f"d1_{gt}_{h}")
            nc.vector.tensor_tensor(d1[:], items[0][:], items[1][:], Add)
            d2 = accp.tile([PT, SH], f16, tag="acc", name=f"d2_{gt}_{h}")
            nc.vector.tensor_tensor(d2[:], items[2][:], items[3][:], Add)
            nc.vector.tensor_tensor(d1[:], d1[:], d2[:], Add)
            z = zpool.tile([PT, SH], f16, tag="z", name=f"z{gt}_{h}")
            nc.vector.tensor_tensor(z[:], ps[:], d1[:], Add)
            z_half[h][gt] = z

        # ---------------- Phase B: one n-tile, one s-half --------------------
        partial_cr = [[p[:].rearrange("(g p) s -> p g s", p=PN)
                       for p in row] for row in partial_c]

        def issue_rs(j, h):
            nc.gpsimd.collective_compute(
                "ReduceScatter", Add,
                replica_groups=[list(range(NCORES))],
                ins=[partial_c[j][h].opt()], outs=[rs_c[j][h].opt()],
            )

        def phase_b(nt, h):
            if h == 0 and nt in w_tiles:
                w = w_tiles.pop(nt)
            else:
                w = load_w0(nt)
            pp = mmps.tile([PN, SH], f32, tag="mm", name=f"mm{nt}_{h}")
            for gt in range(NGT):
                nc.tensor.matmul(pp[:], w[:, gt * PN:(gt + 1) * PN],
                                 z_half[h][gt][:],
                                 start=(gt == 0), stop=(gt == NGT - 1))
            o = opool.tile([PN, SH], f16, tag="o1", name=f"o{nt}_{h}")
            nc.scalar.copy(o[:], pp[:])
            j, ng2 = divmod(nt, NNT // NCH)
            nc.sync.dma_start(partial_cr[j][h][:, ng2, :], o[:])
            if ng2 == NNT // NCH - 1:
                issue_rs(j, h)

        # ---------------- emission schedule ----------------
        # h0 sweep; keep x h0 loads 3 tiles ahead, dg loads 4 ahead
        for gt in range(NGT):
            if gt + 3 < NGT:
                x_h0[gt + 3] = load_x(gt + 3, 0, xpool, nc.gpsimd)
            if gt + 4 < NGT:
                dgn = dgpool.tile([PT, NDIAG * PT], f16, tag="dg",
                                  name=f"dg{gt + 4}")
                nc.gpsimd.dma_start(dgn[:], d["pediag"][gt + 4])
                dg_tiles[gt + 4] = dgn
            phase_a(gt, 0)
            if gt >= NGT - 4:       # prefetch first h1 tiles near sweep end
                x_h1[gt - (NGT - 4)] = load_x(gt - (NGT - 4), 1, x1pool,
                                              nc.gpsimd)

        # h1 sweep interleaved with B(h0) emission (keeps every engine's
        # queue in rough execution order)
        nb = 0
        for gt in range(NGT):
            if gt + 4 < NGT:
                x_h1[gt + 4] = load_x(gt + 4, 1, x1pool, nc.gpsimd)
            phase_a(gt, 1)
            while nb * NGT < (gt + 1) * NNT:
                phase_b(nb, 0)
                nb += 1

        for nt in range(NNT):
            phase_b(nt, 1)

        # ------- Phase D: distributed 2000->200, AllReduce, tail -------
        p2sb = []
        for j in range(NCH):
            y1 = tpool.tile([PN, S], f16, tag=f"y1_{j}", name=f"y1_{j}")
            for h in range(2):
                nc.sync.dma_start(y1[:, h * SH:(h + 1) * SH], rs_c[j][h][:])
            z1 = tpool.tile([PN, S], f16, tag=f"z1_{j}", name=f"z1_{j}")
            nc.scalar.activation(z1[:], y1[:], Relu,
                                 bias=cb0t[:, j:j + 1], scale=1.0)
            p2j = tpool.tile([100, 2 * S], f16, tag=f"p2sb{j}",
                             name=f"p2sb_{j}")
            p2sb.append(p2j)
            for mc in range(2):
                for sh in range(2):
                    ps = mmps.tile([100, SH], f32, tag="mm",
                                   name=f"ps2_{j}{mc}{sh}")
                    nc.tensor.matmul(
                        ps[:], w1t[:, j * N2 + mc * 100:j * N2 + (mc + 1) * 100],
                        z1[:, sh * SH:(sh + 1) * SH], start=True, stop=True)
                    nc.scalar.copy(p2j[:, mc * S + sh * SH:mc * S + (sh + 1) * SH],
                                   ps[:])
        # sum the two chunk partials, single fp16 AllReduce
        nc.vector.tensor_tensor(p2sb[0][:], p2sb[0][:], p2sb[1][:], Add)
        for mc in range(2):
            nc.sync.dma_start(partial2[mc * 100:(mc + 1) * 100, :],
                              p2sb[0][:, mc * S:(mc + 1) * S])
        nc.gpsimd.collective_compute(
            "AllReduce", Add,
            replica_groups=[list(range(NCORES))],
            ins=[partial2.opt()], outs=[summed2.opt()],
        )
        z2all = tpool.tile([100, 2 * S], f16, tag="z2")
        z2_tiles = [z2all[:, 0:S], z2all[:, S:2 * S]]
        for mc in range(2):
            y2a = tpool.tile([100, S], f16, tag=f"y2a_{mc}", name=f"y2a_{mc}")
            nc.sync.dma_start(y2a[:], summed2[mc * 100:(mc + 1) * 100, :])
            nc.scalar.activation(z2_tiles[mc][:], y2a[:], Relu,
                                 bias=cb1t[:, mc:mc + 1], scale=1.0)
        z3 = tpool.tile([N3, S], f16, tag="z3")
        for sh in range(2):
            ps = mmps.tile([N3, SH], f32, tag="mm")
            for mc in range(2):
                nc.tensor.matmul(ps[:], cw2tt[:, mc * N3:(mc + 1) * N3],
                                 z2_tiles[mc][:, sh * SH:(sh + 1) * SH],
                                 start=(mc == 0), stop=(mc == 1))
            nc.scalar.activation(z3[:, sh * SH:(sh + 1) * SH], ps[:], Relu,
                                 bias=cb2t[:], scale=1.0)
        outt = tpool.tile([1, S], f32, tag="outt")
        for sh in range(2):
            ps = mmps.tile([1, SH], f32, tag="mm")
            nc.tensor.matmul(ps[:], cwftt[:],
                             z3[:, sh * SH:(sh + 1) * SH],
                             start=True, stop=True)
            nc.scalar.activation(outt[:, sh * SH:(sh + 1) * SH], ps[:], Ident,
                                 bias=cbft[:], scale=1.0)
        nc.sync.dma_start(out_d[:], outt[:])

    nc.compile()
    return nc


def _shard_inputs(x, W1, b1, W2, b2, Wc, bc,
                  CW0, Cb0, CW1, Cb1, CW2, Cb2, CWf, Cbf):
    f = lambda a: np.ascontiguousarray(a, dtype=np.float32)
    h16 = lambda a: np.ascontiguousarray(a).astype(np.float16)
    CW1T = np.ascontiguousarray(CW1.T)
    shared = {
        "cb1": f(Cb1.reshape(2, 100).T),
        "cw2t": h16(CW2.T),
        "cb2": f(Cb2.reshape(N3, 1)),
        "cwft": h16(CWf.T),
        "cbf": f(Cbf.reshape(1, 1)),
    }

    # c_k = W2 * Wc  (T,G,H); relu-form: c*relu(s x + b) = sgn*relu(|c|s x + |c|b)
    # PE_LANE terms keep raw (s, b) -- the signed c-diagonal applies the weight.
    c = W2.astype(np.float64) * Wc.T.astype(np.float64)[:, :, None]
    ca = np.abs(c)
    scl2 = ca * W1.astype(np.float64)                 # (T,G,H)
    bia2 = ca * b1.astype(np.float64)
    sgn = np.where(c >= 0, 1.0, -1.0)
    for k in PE_LANE:
        t, hh = divmod(k, H)
        scl2[t, :, hh] = W1[t, :, hh]
        bia2[t, :, hh] = b1[t, :, hh]
    # (DVE_LANE keeps |c| folded into scl2/bia2 with sgn applied post-relu)
    # fold: sum_t b2*Wc + bc  -> Cb0
    fold = ((b2.astype(np.float64) * Wc.T.astype(np.float64)).sum(axis=0)
            + bc.astype(np.float64))                  # (G,)
    Cb0_adj = Cb0.astype(np.float64) + CW0.astype(np.float64) @ fold

    def per_gene(arr):   # (T, GL, H) -> (GL, NK), k = t*H+h
        return arr.transpose(1, 0, 2).reshape(GL, NK)

    in_maps = []
    for cix in range(NCORES):
        gs = slice(cix * GL, (cix + 1) * GL)
        scl2c = per_gene(scl2[:, gs, :])
        bia2c = per_gene(bia2[:, gs, :])
        sgnc = per_gene(sgn[:, gs, :])

        # pediag: (NGT, PT, NDIAG*PT): signed c diagonals for PE_LANE terms
        cc = per_gene(c[:, gs, :])
        pediag = np.zeros((NGT, PT, NDIAG * PT), dtype=np.float64)
        idx = np.arange(PT)
        for gt in range(NGT):
            rows = slice(gt * PT, (gt + 1) * PT)
            for j, k in enumerate(PE_LANE):
                pediag[gt, idx, j * PT + idx] = cc[rows, k]

        def col_layout(a):   # (GL, NK) -> (PT, NGT*NK)
            return f(a.reshape(NGT, PT, NK).transpose(1, 0, 2)
                     .reshape(PT, NGT * NK))

        half = N1 // 2
        in_maps.append({
            "xT": np.ascontiguousarray(
                x[:, :, gs].transpose(2, 0, 1).reshape(NGT, PT, T * S)
            ).astype(np.float16),
            "scl2": col_layout(scl2c),
            "bia2": col_layout(bia2c),
            "sgn": col_layout(sgnc),
            "pediag": pediag.astype(np.float16),
            "cw0t": np.ascontiguousarray(
                CW0[:, gs].reshape(NNT, PN, NGT, PT)
                .transpose(0, 3, 2, 1).reshape(NNT, PT, NGT * PN)
            ).astype(np.float16),
            "cb0": f(np.stack([Cb0_adj[cix * PN:(cix + 1) * PN],
                               Cb0_adj[half + cix * PN:half + (cix + 1) * PN]],
                              axis=1)),
            "cw1t": h16(np.concatenate(
                [CW1T[cix * PN:(cix + 1) * PN, :],
                 CW1T[half + cix * PN:half + (cix + 1) * PN, :]], axis=1)),
            **shared,
        })
    return in_maps


def _install_profile_shim():
    """Register the NTFF profiling hook that this container's antenv lacks."""
    import types
    try:
        import antenv.axon_hooks  # noqa: F401
        return True
    except ImportError:
        pass
    try:
        import antenv
        from trn_agent_boot.trn_boot import _ntff_profile_via_ctypes
        hook = _ntff_profile_via_ctypes("/opt/axon/libaxon_pjrt.so")
        mod = types.ModuleType("antenv.axon_hooks")
        mod.get_axon_ntff_profile_hook = lambda: hook
        mod.set_axon_ntff_profile_hook = lambda h: None
        sys.modules["antenv.axon_hooks"] = mod
        antenv.axon_hooks = mod
        return hook is not None
    except Exception:
        return False


def kernel(**inputs):
    inputs = {k: np.asarray(v) for k, v in inputs.items()}
    in_maps = _shard_inputs(**inputs)
    if "nc" not in _CACHE:
        _CACHE["nc"] = _build_program()
    nc = _CACHE["nc"]
    trace = bool(os.environ.get("KERNEL_PROFILE")) and _install_profile_shim()
    res = run_bass_kernel_spmd(nc, in_maps, core_ids=list(range(NCORES)),
                               trace=trace)
    LAST_RUN["exec_time_ns"] = res.exec_time_ns
    LAST_RUN["mean_exec_time_ns"] = res.mean_exec_time_ns
    if res.instructions_and_trace is not None:
        LAST_RUN["trace_path"] = res.instructions_and_trace[1]
    return res.results[0]["out"].reshape(1, S, 1)


if __name__ == "__main__":
    rng = np.random.default_rng(0)
    ins = {
        "x": rng.standard_normal((T, S, G), dtype=np.float32),
        "W1": rng.standard_normal((T, G, H), dtype=np.float32) * 0.5,
        "b1": rng.standard_normal((T, G, H), dtype=np.float32) * 0.1,
        "W2": rng.standard_normal((T, G, H), dtype=np.float32) * 0.5,
        "b2": rng.standard_normal((T, G), dtype=np.float32) * 0.1,
        "Wc": rng.standard_normal((G, T), dtype=np.float32) * 0.5,
        "bc": rng.standard_normal((G,), dtype=np.float32) * 0.1,
        "CW0": rng.standard_normal((N1, G), dtype=np.float32) * 0.007,
        "Cb0": rng.standard_normal((N1,), dtype=np.float32) * 0.007,
        "CW1": rng.standard_normal((N2, N1), dtype=np.float32) * 0.02,
        "Cb1": rng.standard_normal((N2,), dtype=np.float32) * 0.02,
        "CW2": rng.standard_normal((N3, N2), dtype=np.float32) * 0.07,
        "Cb2": rng.standard_normal((N3,), dtype=np.float32) * 0.07,
        "CWf": rng.standard_normal((1, N3), dtype=np.float32) * 0.2,
        "Cbf": rng.standard_normal((1,), dtype=np.float32) * 0.2,
    }
    out = kernel(**ins)
    xx = ins["x"]
    h = np.maximum(xx[..., None] * ins["W1"][:, None] + ins["b1"][:, None], 0.0)
    y = np.einsum("tsgh,tgh->tsg", h, ins["W2"]) + ins["b2"][:, None, :]
    zz = np.einsum("tsg,gt->sg", y, ins["Wc"]) + ins["bc"]
    for Wl, bl in ((ins["CW0"], ins["Cb0"]), (ins["CW1"], ins["Cb1"]),
                   (ins["CW2"], ins["Cb2"])):
        zz = np.maximum(zz @ Wl.T + bl, 0.0)
    ref = (zz @ ins["CWf"].T + ins["Cbf"])[None]
    err = np.abs(out - ref).max() / (np.abs(ref).max() + 1e-12)
    print("self-test rel err:", err)
    print("exec_time_ns:", LAST_RUN.get("exec_time_ns"))



# revision 3
# speedup vs baseline: 1.0421x; 1.0421x over previous
"""Trainium2 Bass kernel for nn_CombinedAMLModel (dense_mlp, 8 NeuronCores).

Sharding: tensor-parallel over the gene axis (20000 genes -> 2500 per core).

Math: per (tech,gene) 1->4->1 MLP + per-gene tech combinor gives, per gene g
and sample s:
    z[g,s] = sum_k c_k * relu(s_k x_t + b_k) + const        (k = (t,h), 12 terms)
with c_k*relu(u) = sgn(c_k)*relu(|c_k| u). The per-gene const is folded into
Cb0 on the host (z feeds CW0 linearly), so the device computes only the
12 signed relu terms.

Per core, s-half pipelined (h0 sweep -> B(h0) || h1 sweep -> B(h1)):
  Phase A (per gene tile of 125, per s-half of 512):
    - 7 relu terms on ACT (fp16), 5 on DVE (two 4x tensor_scalar ops,
      relu+sign folded into the second op); signs for ACT's non-PE terms
      applied by DVE 4x multiplies.
    - 3 of the ACT terms (raw relu) accumulate in PSUM via host-built
      c-diagonal matmuls (fp16, signed weights).
    - the other 9 signed terms accumulate via a DVE/Pool tensor_tensor tree;
      DVE merges psum + sbuf accumulator -> z fp16.
  Phase B: out1[nt] += CW0^T-block @ z (fp16 matmuls, K=125 per step,
    PSUM accumulation over 20 gene tiles), PSUM->fp16 staging on Pool,
    DMA to DRAM partials, chunked fp16 ReduceScatter overlapped.
  Phase D: distributed 2000->200 (each core owns its ReduceScatter slice),
    fp16 AllReduce of the (200,1024) partial, then 200->20->1 replicated.
"""
import os
import sys

sys.path.insert(0, "/opt/trn_rl_repo")

import ml_dtypes
import numpy as np
from contextlib import ExitStack

import concourse.bass as bass
import concourse.tile as tile
from concourse import bacc, mybir
from concourse.bass_utils import run_bass_kernel_spmd

T, S, G, H = 3, 1024, 20000, 4
NCORES = 8
GL = G // NCORES            # genes per core
PT = 125                    # gene-tile partition size
NGT = GL // PT              # gene tiles per core
NK = T * H                  # local terms per gene
N1, N2, N3 = 2000, 200, 20
PN = 125                    # n-tile partition size for layer-1 output
NNT = N1 // PN              # n tiles
SH = 512                    # s-half size (PSUM bank)

# term assignment: k = t*H + h
GEN_ACT = (0, 1, 2, 3, 4, 5, 6)      # generated on ACT (Relu activation)
GEN_DVE = (7, 8, 9, 10, 11)          # generated on DVE (2x tensor_scalar pair)
PE_LANE = (0, 1, 2, 3, 4, 5, 6, 7)   # terms accumulated via c-diag matmul
DVE_LANE = (8, 9, 10, 11)            # signed terms accumulated via DVE adds
NDIAG = len(PE_LANE)                 # diag blocks per tile

f32 = mybir.dt.float32
f16 = mybir.dt.float16
Add = mybir.AluOpType.add
Mult = mybir.AluOpType.mult
Max = mybir.AluOpType.max

LAST_RUN = {}
_CACHE = {}


def _build_program():
    nc = bacc.Bacc("TRN2", target_bir_lowering=False, debug=False,
                   num_devices=NCORES)
    d = {}

    def inp(name, shape, dt=f32):
        d[name] = nc.dram_tensor(name, list(shape), dt, kind="ExternalInput").ap()

    inp("xT", (NGT, PT, T * S), f16)
    inp("scl2", (PT, NGT * NK))
    inp("bia2", (PT, NGT * NK))
    inp("sgn", (PT, NGT * NK))
    inp("pediag", (NGT, PT, NDIAG * PT), f16)
    inp("cw0t", (NNT, PT, NGT * PN), f16)
    inp("cb0", (PN, 2))
    inp("cw1t", (PN, 2 * N2), f16)
    inp("cb1", (100, 2))
    inp("cw2t", (N2, N3), f16)
    inp("cb2", (N3, 1))
    inp("cwft", (N3, 1), f16)
    inp("cbf", (1, 1))
    out_d = nc.dram_tensor("out", [1, S], f32, kind="ExternalOutput").ap()

    Relu = mybir.ActivationFunctionType.Relu
    Ident = mybir.ActivationFunctionType.Identity
    Abs = mybir.ActivationFunctionType.Abs

    with tile.TileContext(nc) as tc, ExitStack() as ctx:
        const = ctx.enter_context(tc.tile_pool(name="const", bufs=1))
        xpool = ctx.enter_context(tc.tile_pool(name="x", bufs=6))
        x1pool = ctx.enter_context(tc.tile_pool(name="x1", bufs=9))
        dgpool = ctx.enter_context(tc.tile_pool(name="dg", bufs=NGT))
        tpool_a = ctx.enter_context(tc.tile_pool(name="ta", bufs=8))
        tpool_u = ctx.enter_context(tc.tile_pool(name="tu", bufs=6))
        tpool_v = ctx.enter_context(tc.tile_pool(name="tv", bufs=8))
        accp = ctx.enter_context(tc.tile_pool(name="acc", bufs=4))
        zpool = ctx.enter_context(tc.tile_pool(name="z", bufs=2 * NGT))
        wpool = ctx.enter_context(tc.tile_pool(name="w0", bufs=3))
        opool = ctx.enter_context(tc.tile_pool(name="o1", bufs=4))
        tpool = ctx.enter_context(tc.tile_pool(name="tail", bufs=1))
        zps = ctx.enter_context(tc.tile_pool(name="zps", bufs=3, space="PSUM"))
        mmps = ctx.enter_context(tc.tile_pool(name="mmps", bufs=4, space="PSUM"))
        dram = ctx.enter_context(tc.tile_pool(name="dram", bufs=1, space="DRAM"))

        # ---- DRAM scratch ----
        NCH = 2
        partial_c = [[dram.tile([N1 // NCH, SH], f16, tag=f"pc{j}{h}",
                                name=f"partial{j}_{h}") for h in range(2)]
                     for j in range(NCH)]
        rs_c = [[dram.tile([PN, SH], f16, tag=f"rs{j}{h}",
                           name=f"rsout{j}_{h}") for h in range(2)]
                for j in range(NCH)]
        partial2 = dram.tile([N2, S], f16, tag="p2", name="partial2")
        summed2 = dram.tile([N2, S], f16, tag="s2", name="summed2")
        ccwarm_in = dram.tile([1, 128], f32, tag="ccwi")
        ccwarm_out = dram.tile([1, 128], f32, tag="ccwo")

        # tiny warm-up collective (absorbs first-rendezvous cost early)
        ccwarm_sb = const.tile([1, 128], f32)
        nc.gpsimd.memset(ccwarm_sb[:], 0.0)
        nc.gpsimd.dma_start(ccwarm_in[:], ccwarm_sb[:])
        nc.gpsimd.collective_compute(
            "AllReduce", Add,
            replica_groups=[list(range(NCORES))],
            ins=[ccwarm_in.opt()], outs=[ccwarm_out.opt()],
        )

        # ---- x loads: tile gt, half h -> (PT, T, SH) fp16 ----
        x_h0 = {}
        x_h1 = {}

        def load_x(gt, h, pool, eng):
            xt = pool.tile([PT, T, SH], f16, tag="x" if h == 0 else "x1",
                           name=f"x{gt}_{h}")
            src = d["xT"][gt].rearrange("p (t s) -> p t s", t=T)
            eng.dma_start(xt[:], src[:, :, h * SH:(h + 1) * SH])
            return xt

        # first few h0 tiles before anything else (gate the A ramp)
        for gt in range(3):
            x_h0[gt] = load_x(gt, 0, xpool, nc.gpsimd)

        # ---- consts ----
        scl2t = const.tile([PT, NGT * NK], f32)
        nc.scalar.dma_start(scl2t[:], d["scl2"][:])
        bia2t = const.tile([PT, NGT * NK], f32)
        nc.scalar.dma_start(bia2t[:], d["bia2"][:])
        sgnt = const.tile([PT, NGT * NK], f32)
        nc.scalar.dma_start(sgnt[:], d["sgn"][:])
        cb0t = const.tile([PN, 2], f32)
        nc.scalar.dma_start(cb0t[:], d["cb0"][:])
        w1t = const.tile([PN, 2 * N2], f16)
        nc.scalar.dma_start(w1t[:], d["cw1t"][:])
        cb1t = const.tile([100, 2], f32)
        nc.scalar.dma_start(cb1t[:], d["cb1"][:])
        cb2t = const.tile([N3, 1], f32)
        nc.scalar.dma_start(cb2t[:], d["cb2"][:])
        cwftt = const.tile([N3, 1], f16)
        nc.scalar.dma_start(cwftt[:], d["cwft"][:])
        cbft = const.tile([1, 1], f32)
        nc.scalar.dma_start(cbft[:], d["cbf"][:])
        cw2tt = const.tile([100, 2 * N3], f16)
        for mc in range(2):
            nc.scalar.dma_start(cw2tt[:, mc * N3:(mc + 1) * N3],
                                d["cw2t"][mc * 100:(mc + 1) * 100, :])

        # per-tile diagonal blocks (3 sgn + 3 alpha), live for both halves
        dg_tiles = {}
        for gt in range(4):
            dg = dgpool.tile([PT, NDIAG * PT], f16, tag="dg", name=f"dg{gt}")
            nc.gpsimd.dma_start(dg[:], d["pediag"][gt])
            dg_tiles[gt] = dg

        # cw0 prefetch: first blocks load during phase A
        HW0 = NGT * PN // 2

        def load_w0(nt):
            w = wpool.tile([PT, NGT * PN], f16, tag="w0", name=f"w0_{nt}")
            nc.gpsimd.dma_start(w[:], d["cw0t"][nt])
            return w

        w_tiles = {nt: load_w0(nt) for nt in range(2)}

        # ---------------- Phase A: one gene tile, one s-half ----------------
        z_half = [[None] * NGT, [None] * NGT]

        def phase_a(gt, h):
            xh = (x_h0 if h == 0 else x_h1)[gt]
            dg = dg_tiles[gt]
            col = lambda k: slice(gt * NK + k, gt * NK + k + 1)
            xs = lambda k: xh[:, k // H, :]

            # ACT terms (PE-lane, raw relu): relu(s*x + b) in fp16
            terms = {}
            for k in GEN_ACT:
                a = tpool_a.tile([PT, SH], f16, tag="ta", name=f"a{gt}_{h}_{k}")
                nc.scalar.activation(a[:], xs(k), Relu,
                                     bias=bia2t[:, col(k)],
                                     scale=scl2t[:, col(k)])
                terms[k] = a

            # DVE terms: u = scl2*x + bia2 ; then relu (PE-lane) or
            # relu*sgn (DVE-lane) -- both 2x fp16 tensor_scalar
            for k in GEN_DVE:
                u = tpool_u.tile([PT, SH], f16, tag="tu", name=f"u{gt}_{h}_{k}")
                nc.vector.tensor_scalar(u[:], xs(k), scl2t[:, col(k)],
                                        bia2t[:, col(k)], Mult, Add)
                v = tpool_v.tile([PT, SH], f16, tag="tv", name=f"v{gt}_{h}_{k}")
                if k in PE_LANE:
                    nc.vector.tensor_scalar(v[:], u[:], 0.0, None, Max)
                else:
                    nc.vector.tensor_scalar(v[:], u[:], 0.0, sgnt[:, col(k)],
                                            Max, Mult)
                terms[k] = v

            # PSUM: 8 c-diag weighted term accumulations
            ps = zps.tile([PT, SH], f32, tag="zps", name=f"zps{gt}_{h}")
            for j, k in enumerate(PE_LANE):
                nc.tensor.matmul(ps[:], dg[:, j * PT:(j + 1) * PT],
                                 terms[k][:], start=(j == 0),
                                 stop=(j == len(PE_LANE) - 1))

            # DVE-lane: 3 adds + psum merge -> z fp16
            items = [terms[k] for k in DVE_LANE]
            d1 = accp.tile([PT, SH], f16, tag="acc", name=f"d1_{gt}_{h}")
            nc.vector.tensor_tensor(d1[:], items[0][:], items[1][:], Add)
            d2 = accp.tile([PT, SH], f16, tag="acc", name=f"d2_{gt}_{h}")
            nc.vector.tensor_tensor(d2[:], items[2][:], items[3][:], Add)
            nc.vector.tensor_tensor(d1[:], d1[:], d2[:], Add)
            z = zpool.tile([PT, SH], f16, tag="z", name=f"z{gt}_{h}")
            nc.vector.tensor_tensor(z[:], ps[:], d1[:], Add)
            z_half[h][gt] = z

        # ---------------- Phase B: one n-tile, one s-half --------------------
        partial_cr = [[p[:].rearrange("(g p) s -> p g s", p=PN)
                       for p in row] for row in partial_c]

        def issue_rs(j, h):
            nc.gpsimd.collective_compute(
                "ReduceScatter", Add,
                replica_groups=[list(range(NCORES))],
                ins=[partial_c[j][h].opt()], outs=[rs_c[j][h].opt()],
            )

        def phase_b(nt, h):
            if h == 0 and nt in w_tiles:
                w = w_tiles.pop(nt)
            else:
                w = load_w0(nt)
            pp = mmps.tile([PN, SH], f32, tag="mm", name=f"mm{nt}_{h}")
            for gt in range(NGT):
                nc.tensor.matmul(pp[:], w[:, gt * PN:(gt + 1) * PN],
                                 z_half[h][gt][:],
                                 start=(gt == 0), stop=(gt == NGT - 1))
            o = opool.tile([PN, SH], f16, tag="o1", name=f"o{nt}_{h}")
            nc.scalar.copy(o[:], pp[:])
            j, ng2 = divmod(nt, NNT // NCH)
            nc.sync.dma_start(partial_cr[j][h][:, ng2, :], o[:])
            if ng2 == NNT // NCH - 1:
                issue_rs(j, h)

        # ---------------- emission schedule ----------------
        # h0 sweep; keep x h0 loads 3 tiles ahead, dg loads 4 ahead
        for gt in range(NGT):
            if gt + 3 < NGT:
                x_h0[gt + 3] = load_x(gt + 3, 0, xpool, nc.gpsimd)
            if gt + 4 < NGT:
                dgn = dgpool.tile([PT, NDIAG * PT], f16, tag="dg",
                                  name=f"dg{gt + 4}")
                nc.gpsimd.dma_start(dgn[:], d["pediag"][gt + 4])
                dg_tiles[gt + 4] = dgn
            phase_a(gt, 0)
            if gt >= NGT - 8:       # prefetch first h1 tiles early (8 ahead)
                x_h1[gt - (NGT - 8)] = load_x(gt - (NGT - 8), 1, x1pool,
                                              nc.sync)

        # h1 sweep interleaved with B(h0) emission (keeps every engine's
        # queue in rough execution order)
        nb = 0
        for gt in range(NGT):
            if gt + 8 < NGT:
                x_h1[gt + 8] = load_x(gt + 8, 1, x1pool, nc.sync)
            phase_a(gt, 1)
            while nb * NGT < (gt + 1) * NNT:
                phase_b(nb, 0)
                nb += 1

        for nt in range(NNT):
            phase_b(nt, 1)

        # ------- Phase D: distributed 2000->200, AllReduce, tail -------
        p2sb = []
        for j in range(NCH):
            y1 = tpool.tile([PN, S], f16, tag=f"y1_{j}", name=f"y1_{j}")
            for h in range(2):
                nc.sync.dma_start(y1[:, h * SH:(h + 1) * SH], rs_c[j][h][:])
            z1 = tpool.tile([PN, S], f16, tag=f"z1_{j}", name=f"z1_{j}")
            nc.scalar.activation(z1[:], y1[:], Relu,
                                 bias=cb0t[:, j:j + 1], scale=1.0)
            p2j = tpool.tile([100, 2 * S], f16, tag=f"p2sb{j}",
                             name=f"p2sb_{j}")
            p2sb.append(p2j)
            for mc in range(2):
                for sh in range(2):
                    ps = mmps.tile([100, SH], f32, tag="mm",
                                   name=f"ps2_{j}{mc}{sh}")
                    nc.tensor.matmul(
                        ps[:], w1t[:, j * N2 + mc * 100:j * N2 + (mc + 1) * 100],
                        z1[:, sh * SH:(sh + 1) * SH], start=True, stop=True)
                    nc.scalar.copy(p2j[:, mc * S + sh * SH:mc * S + (sh + 1) * SH],
                                   ps[:])
        # sum the two chunk partials, single fp16 AllReduce
        nc.vector.tensor_tensor(p2sb[0][:], p2sb[0][:], p2sb[1][:], Add)
        for mc in range(2):
            nc.sync.dma_start(partial2[mc * 100:(mc + 1) * 100, :],
                              p2sb[0][:, mc * S:(mc + 1) * S])
        nc.gpsimd.collective_compute(
            "AllReduce", Add,
            replica_groups=[list(range(NCORES))],
            ins=[partial2.opt()], outs=[summed2.opt()],
        )
        z2all = tpool.tile([100, 2 * S], f16, tag="z2")
        z2_tiles = [z2all[:, 0:S], z2all[:, S:2 * S]]
        for mc in range(2):
            y2a = tpool.tile([100, S], f16, tag=f"y2a_{mc}", name=f"y2a_{mc}")
            nc.sync.dma_start(y2a[:], summed2[mc * 100:(mc + 1) * 100, :])
            nc.scalar.activation(z2_tiles[mc][:], y2a[:], Relu,
                                 bias=cb1t[:, mc:mc + 1], scale=1.0)
        z3 = tpool.tile([N3, S], f16, tag="z3")
        for sh in range(2):
            ps = mmps.tile([N3, SH], f32, tag="mm")
            for mc in range(2):
                nc.tensor.matmul(ps[:], cw2tt[:, mc * N3:(mc + 1) * N3],
                                 z2_tiles[mc][:, sh * SH:(sh + 1) * SH],
                                 start=(mc == 0), stop=(mc == 1))
            nc.scalar.activation(z3[:, sh * SH:(sh + 1) * SH], ps[:], Relu,
                                 bias=cb2t[:], scale=1.0)
        outt = tpool.tile([1, S], f32, tag="outt")
        for sh in range(2):
            ps = mmps.tile([1, SH], f32, tag="mm")
            nc.tensor.matmul(ps[:], cwftt[:],
                             z3[:, sh * SH:(sh + 1) * SH],
                             start=True, stop=True)
            nc.scalar.activation(outt[:, sh * SH:(sh + 1) * SH], ps[:], Ident,
                                 bias=cbft[:], scale=1.0)
        nc.sync.dma_start(out_d[:], outt[:])

    nc.compile()
    return nc


def _shard_inputs(x, W1, b1, W2, b2, Wc, bc,
                  CW0, Cb0, CW1, Cb1, CW2, Cb2, CWf, Cbf):
    f = lambda a: np.ascontiguousarray(a, dtype=np.float32)
    h16 = lambda a: np.ascontiguousarray(a).astype(np.float16)
    CW1T = np.ascontiguousarray(CW1.T)
    shared = {
        "cb1": f(Cb1.reshape(2, 100).T),
        "cw2t": h16(CW2.T),
        "cb2": f(Cb2.reshape(N3, 1)),
        "cwft": h16(CWf.T),
        "cbf": f(Cbf.reshape(1, 1)),
    }

    # c_k = W2 * Wc  (T,G,H); relu-form: c*relu(s x + b) = sgn*relu(|c|s x + |c|b)
    # PE_LANE terms keep raw (s, b) -- the signed c-diagonal applies the weight.
    c = W2.astype(np.float64) * Wc.T.astype(np.float64)[:, :, None]
    ca = np.abs(c)
    scl2 = ca * W1.astype(np.float64)                 # (T,G,H)
    bia2 = ca * b1.astype(np.float64)
    sgn = np.where(c >= 0, 1.0, -1.0)
    for k in PE_LANE:
        t, hh = divmod(k, H)
        scl2[t, :, hh] = W1[t, :, hh]
        bia2[t, :, hh] = b1[t, :, hh]
    # (DVE_LANE keeps |c| folded into scl2/bia2 with sgn applied post-relu)
    # fold: sum_t b2*Wc + bc  -> Cb0
    fold = ((b2.astype(np.float64) * Wc.T.astype(np.float64)).sum(axis=0)
            + bc.astype(np.float64))                  # (G,)
    Cb0_adj = Cb0.astype(np.float64) + CW0.astype(np.float64) @ fold

    def per_gene(arr):   # (T, GL, H) -> (GL, NK), k = t*H+h
        return arr.transpose(1, 0, 2).reshape(GL, NK)

    in_maps = []
    for cix in range(NCORES):
        gs = slice(cix * GL, (cix + 1) * GL)
        scl2c = per_gene(scl2[:, gs, :])
        bia2c = per_gene(bia2[:, gs, :])
        sgnc = per_gene(sgn[:, gs, :])

        # pediag: (NGT, PT, NDIAG*PT): signed c diagonals for PE_LANE terms
        cc = per_gene(c[:, gs, :])
        pediag = np.zeros((NGT, PT, NDIAG * PT), dtype=np.float64)
        idx = np.arange(PT)
        for gt in range(NGT):
            rows = slice(gt * PT, (gt + 1) * PT)
            for j, k in enumerate(PE_LANE):
                pediag[gt, idx, j * PT + idx] = cc[rows, k]

        def col_layout(a):   # (GL, NK) -> (PT, NGT*NK)
            return f(a.reshape(NGT, PT, NK).transpose(1, 0, 2)
                     .reshape(PT, NGT * NK))

        half = N1 // 2
        in_maps.append({
            "xT": np.ascontiguousarray(
                x[:, :, gs].transpose(2, 0, 1).reshape(NGT, PT, T * S)
            ).astype(np.float16),
            "scl2": col_layout(scl2c),
            "bia2": col_layout(bia2c),
            "sgn": col_layout(sgnc),
            "pediag": pediag.astype(np.float16),
            "cw0t": np.ascontiguousarray(
                CW0[:, gs].reshape(NNT, PN, NGT, PT)
                .transpose(0, 3, 2, 1).reshape(NNT, PT, NGT * PN)
            ).astype(np.float16),
            "cb0": f(np.stack([Cb0_adj[cix * PN:(cix + 1) * PN],
                               Cb0_adj[half + cix * PN:half + (cix + 1) * PN]],
                              axis=1)),
            "cw1t": h16(np.concatenate(
                [CW1T[cix * PN:(cix + 1) * PN, :],
                 CW1T[half + cix * PN:half + (cix + 1) * PN, :]], axis=1)),
            **shared,
        })
    return in_maps


def _install_profile_shim():
    """Register the NTFF profiling hook that this container's antenv lacks."""
    import types
    try:
        import antenv.axon_hooks  # noqa: F401
        return True
    except ImportError:
        pass
    try:
        import antenv
        from trn_agent_boot.trn_boot import _ntff_profile_via_ctypes
        hook = _ntff_profile_via_ctypes("/opt/axon/libaxon_pjrt.so")
        mod = types.ModuleType("antenv.axon_hooks")
        mod.get_axon_ntff_profile_hook = lambda: hook
        mod.set_axon_ntff_profile_hook = lambda h: None
        sys.modules["antenv.axon_hooks"] = mod
        antenv.axon_hooks = mod
        return hook is not None
    except Exception:
        return False


def kernel(**inputs):
    inputs = {k: np.asarray(v) for k, v in inputs.items()}
    in_maps = _shard_inputs(**inputs)
    if "nc" not in _CACHE:
        _CACHE["nc"] = _build_program()
    nc = _CACHE["nc"]
    trace = bool(os.environ.get("KERNEL_PROFILE")) and _install_profile_shim()
    res = run_bass_kernel_spmd(nc, in_maps, core_ids=list(range(NCORES)),
                               trace=trace)
    LAST_RUN["exec_time_ns"] = res.exec_time_ns
    LAST_RUN["mean_exec_time_ns"] = res.mean_exec_time_ns
    if res.instructions_and_trace is not None:
        LAST_RUN["trace_path"] = res.instructions_and_trace[1]
    return res.results[0]["out"].reshape(1, S, 1)


if __name__ == "__main__":
    rng = np.random.default_rng(0)
    ins = {
        "x": rng.standard_normal((T, S, G), dtype=np.float32),
        "W1": rng.standard_normal((T, G, H), dtype=np.float32) * 0.5,
        "b1": rng.standard_normal((T, G, H), dtype=np.float32) * 0.1,
        "W2": rng.standard_normal((T, G, H), dtype=np.float32) * 0.5,
        "b2": rng.standard_normal((T, G), dtype=np.float32) * 0.1,
        "Wc": rng.standard_normal((G, T), dtype=np.float32) * 0.5,
        "bc": rng.standard_normal((G,), dtype=np.float32) * 0.1,
        "CW0": rng.standard_normal((N1, G), dtype=np.float32) * 0.007,
        "Cb0": rng.standard_normal((N1,), dtype=np.float32) * 0.007,
        "CW1": rng.standard_normal((N2, N1), dtype=np.float32) * 0.02,
        "Cb1": rng.standard_normal((N2,), dtype=np.float32) * 0.02,
        "CW2": rng.standard_normal((N3, N2), dtype=np.float32) * 0.07,
        "Cb2": rng.standard_normal((N3,), dtype=np.float32) * 0.07,
        "CWf": rng.standard_normal((1, N3), dtype=np.float32) * 0.2,
        "Cbf": rng.standard_normal((1,), dtype=np.float32) * 0.2,
    }
    out = kernel(**ins)
    xx = ins["x"]
    h = np.maximum(xx[..., None] * ins["W1"][:, None] + ins["b1"][:, None], 0.0)
    y = np.einsum("tsgh,tgh->tsg", h, ins["W2"]) + ins["b2"][:, None, :]
    zz = np.einsum("tsg,gt->sg", y, ins["Wc"]) + ins["bc"]
    for Wl, bl in ((ins["CW0"], ins["Cb0"]), (ins["CW1"], ins["Cb1"]),
                   (ins["CW2"], ins["Cb2"])):
        zz = np.maximum(zz @ Wl.T + bl, 0.0)
    ref = (zz @ ins["CWf"].T + ins["Cbf"])[None]
    err = np.abs(out - ref).max() / (np.abs(ref).max() + 1e-12)
    print("self-test rel err:", err)
    print("exec_time_ns:", LAST_RUN.get("exec_time_ns"))



# revision 4
# speedup vs baseline: 1.0824x; 1.0387x over previous
"""Trainium2 Bass kernel for nn_CombinedAMLModel (dense_mlp, 8 NeuronCores).

Sharding: tensor-parallel over the gene axis (20000 genes -> 2500 per core).

Math: per (tech,gene) 1->4->1 MLP + per-gene tech combinor gives, per gene g
and sample s:
    z[g,s] = sum_k c_k * relu(s_k x_t + b_k) + const        (k = (t,h), 12 terms)
with c_k*relu(u) = sgn(c_k)*relu(|c_k| u). The per-gene const is folded into
Cb0 on the host (z feeds CW0 linearly), so the device computes only the
12 signed relu terms.

Per core, s-half pipelined (h0 sweep -> B(h0) || h1 sweep -> B(h1)):
  Phase A (per gene tile of 125, per s-half of 512):
    - 7 relu terms on ACT (fp16), 5 on DVE (two 4x tensor_scalar ops,
      relu+sign folded into the second op); signs for ACT's non-PE terms
      applied by DVE 4x multiplies.
    - 3 of the ACT terms (raw relu) accumulate in PSUM via host-built
      c-diagonal matmuls (fp16, signed weights).
    - the other 9 signed terms accumulate via a DVE/Pool tensor_tensor tree;
      DVE merges psum + sbuf accumulator -> z fp16.
  Phase B: out1[nt] += CW0^T-block @ z (fp16 matmuls, K=125 per step,
    PSUM accumulation over 20 gene tiles), PSUM->fp16 staging on Pool,
    DMA to DRAM partials, chunked fp16 ReduceScatter overlapped.
  Phase D: distributed 2000->200 (each core owns its ReduceScatter slice),
    fp16 AllReduce of the (200,1024) partial, then 200->20->1 replicated.
"""
import os
import sys

sys.path.insert(0, "/opt/trn_rl_repo")

import ml_dtypes
import numpy as np
from contextlib import ExitStack

import concourse.bass as bass
import concourse.tile as tile
from concourse import bacc, mybir
from concourse.bass_utils import run_bass_kernel_spmd

T, S, G, H = 3, 1024, 20000, 4
NCORES = 8
GL = G // NCORES            # genes per core
PT = 125                    # gene-tile partition size
NGT = GL // PT              # gene tiles per core
NK = T * H                  # local terms per gene
N1, N2, N3 = 2000, 200, 20
PN = 125                    # n-tile partition size for layer-1 output
NNT = N1 // PN              # n tiles
SH = 512                    # s-half size (PSUM bank)

# term assignment: k = t*H + h
GEN_ACT = (0, 1, 2, 3, 4, 5, 6)      # generated on ACT (Relu activation)
GEN_DVE = (7, 8, 9, 10, 11)          # generated on DVE (2x tensor_scalar pair)
PE_LANE = (0, 1, 2, 3, 4, 5, 6, 7)   # terms accumulated via c-diag matmul
DVE_LANE = (8, 9, 10, 11)            # signed terms accumulated via DVE adds
NDIAG = len(PE_LANE)                 # diag blocks per tile

f32 = mybir.dt.float32
f16 = mybir.dt.float16
Add = mybir.AluOpType.add
Mult = mybir.AluOpType.mult
Max = mybir.AluOpType.max

LAST_RUN = {}
_CACHE = {}


def _build_program():
    nc = bacc.Bacc("TRN2", target_bir_lowering=False, debug=False,
                   num_devices=NCORES)
    d = {}

    def inp(name, shape, dt=f32):
        d[name] = nc.dram_tensor(name, list(shape), dt, kind="ExternalInput").ap()

    inp("xT", (NGT, PT, T * S), f16)
    inp("scl2", (PT, NGT * NK))
    inp("bia2", (PT, NGT * NK))
    inp("sgn", (PT, NGT * NK))
    inp("pediag", (NGT, PT, NDIAG * PT), f16)
    inp("cw0t", (NNT, PT, NGT * PN), f16)
    inp("cb0", (PN, 2))
    inp("cw1t", (PN, 2 * N2), f16)
    inp("cb1", (100, 2))
    inp("cw2t", (N2, N3), f16)
    inp("cb2", (N3, 1))
    inp("cwft", (N3, 1), f16)
    inp("cbf", (1, 1))
    out_d = nc.dram_tensor("out", [1, S], f32, kind="ExternalOutput").ap()

    Relu = mybir.ActivationFunctionType.Relu
    Ident = mybir.ActivationFunctionType.Identity
    Abs = mybir.ActivationFunctionType.Abs

    with tile.TileContext(nc) as tc, ExitStack() as ctx:
        const = ctx.enter_context(tc.tile_pool(name="const", bufs=1))
        xpool = ctx.enter_context(tc.tile_pool(name="x", bufs=6))
        x1pool = ctx.enter_context(tc.tile_pool(name="x1", bufs=8))
        dgpool = ctx.enter_context(tc.tile_pool(name="dg", bufs=NGT))
        tpool_a = ctx.enter_context(tc.tile_pool(name="ta", bufs=8))
        tpool_u = ctx.enter_context(tc.tile_pool(name="tu", bufs=6))
        tpool_v = ctx.enter_context(tc.tile_pool(name="tv", bufs=8))
        accp = ctx.enter_context(tc.tile_pool(name="acc", bufs=4))
        zpool = ctx.enter_context(tc.tile_pool(name="z", bufs=2 * NGT))
        wpool = ctx.enter_context(tc.tile_pool(name="w0", bufs=3))
        opool = ctx.enter_context(tc.tile_pool(name="o1", bufs=4))
        tpool = ctx.enter_context(tc.tile_pool(name="tail", bufs=1))
        zps = ctx.enter_context(tc.tile_pool(name="zps", bufs=3, space="PSUM"))
        mmps = ctx.enter_context(tc.tile_pool(name="mmps", bufs=4, space="PSUM"))
        dram = ctx.enter_context(tc.tile_pool(name="dram", bufs=1, space="DRAM"))

        # ---- DRAM scratch ----
        NCH = 2
        partial_c = [[dram.tile([N1 // NCH, SH], f16, tag=f"pc{j}{h}",
                                name=f"partial{j}_{h}") for h in range(2)]
                     for j in range(NCH)]
        rs_c = [[dram.tile([PN, SH], f16, tag=f"rs{j}{h}",
                           name=f"rsout{j}_{h}") for h in range(2)]
                for j in range(NCH)]
        partial2 = dram.tile([N2, S], f16, tag="p2", name="partial2")
        summed2 = dram.tile([N2, S], f16, tag="s2", name="summed2")
        ccwarm_in = dram.tile([1, 128], f32, tag="ccwi")
        ccwarm_out = dram.tile([1, 128], f32, tag="ccwo")

        # tiny warm-up collective (absorbs first-rendezvous cost early)
        ccwarm_sb = const.tile([1, 128], f32)
        nc.gpsimd.memset(ccwarm_sb[:], 0.0)
        nc.gpsimd.dma_start(ccwarm_in[:], ccwarm_sb[:])
        nc.gpsimd.collective_compute(
            "AllReduce", Add,
            replica_groups=[list(range(NCORES))],
            ins=[ccwarm_in.opt()], outs=[ccwarm_out.opt()],
        )

        # ---- x loads: tile gt, half h -> (PT, T, SH) fp16 ----
        x_h0 = {}
        x_h1 = {}

        def load_x(gt, h, pool, eng):
            xt = pool.tile([PT, T, SH], f16, tag="x" if h == 0 else "x1",
                           name=f"x{gt}_{h}")
            src = d["xT"][gt].rearrange("p (t s) -> p t s", t=T)
            eng.dma_start(xt[:], src[:, :, h * SH:(h + 1) * SH])
            return xt

        # first few h0 tiles before anything else (gate the A ramp)
        for gt in range(3):
            x_h0[gt] = load_x(gt, 0, xpool, nc.gpsimd)

        # ---- consts ----
        scl2t = const.tile([PT, NGT * NK], f32)
        nc.scalar.dma_start(scl2t[:], d["scl2"][:])
        bia2t = const.tile([PT, NGT * NK], f32)
        nc.scalar.dma_start(bia2t[:], d["bia2"][:])
        sgnt = const.tile([PT, NGT * NK], f32)
        nc.scalar.dma_start(sgnt[:], d["sgn"][:])
        cb0t = const.tile([PN, 2], f32)
        nc.scalar.dma_start(cb0t[:], d["cb0"][:])
        w1t = const.tile([PN, 2 * N2], f16)
        nc.scalar.dma_start(w1t[:], d["cw1t"][:])
        cb1t = const.tile([100, 2], f32)
        nc.scalar.dma_start(cb1t[:], d["cb1"][:])
        cb2t = const.tile([N3, 1], f32)
        nc.scalar.dma_start(cb2t[:], d["cb2"][:])
        cwftt = const.tile([N3, 1], f16)
        nc.scalar.dma_start(cwftt[:], d["cwft"][:])
        cbft = const.tile([1, 1], f32)
        nc.scalar.dma_start(cbft[:], d["cbf"][:])
        cw2tt = const.tile([100, 2 * N3], f16)
        for mc in range(2):
            nc.scalar.dma_start(cw2tt[:, mc * N3:(mc + 1) * N3],
                                d["cw2t"][mc * 100:(mc + 1) * 100, :])

        # per-tile diagonal blocks (3 sgn + 3 alpha), live for both halves
        dg_tiles = {}
        for gt in range(4):
            dg = dgpool.tile([PT, NDIAG * PT], f16, tag="dg", name=f"dg{gt}")
            nc.gpsimd.dma_start(dg[:], d["pediag"][gt])
            dg_tiles[gt] = dg

        # cw0 prefetch: first blocks load during phase A
        HW0 = NGT * PN // 2

        def load_w0(nt):
            w = wpool.tile([PT, NGT * PN], f16, tag="w0", name=f"w0_{nt}")
            nc.gpsimd.dma_start(w[:], d["cw0t"][nt])
            return w

        w_tiles = {nt: load_w0(nt) for nt in range(2)}

        # ---------------- Phase A: one gene tile, one s-half ----------------
        z_half = [[None] * NGT, [None] * NGT]

        def phase_a(gt, h):
            xh = (x_h0 if h == 0 else x_h1)[gt]
            dg = dg_tiles[gt]
            col = lambda k: slice(gt * NK + k, gt * NK + k + 1)
            xs = lambda k: xh[:, k // H, :]

            # ACT terms (PE-lane, raw relu): relu(s*x + b) in fp16
            terms = {}
            for k in GEN_ACT:
                a = tpool_a.tile([PT, SH], f16, tag="ta", name=f"a{gt}_{h}_{k}")
                nc.scalar.activation(a[:], xs(k), Relu,
                                     bias=bia2t[:, col(k)],
                                     scale=scl2t[:, col(k)])
                terms[k] = a

            # DVE terms: u = scl2*x + bia2 ; then relu (PE-lane) or
            # relu*sgn (DVE-lane) -- both 2x fp16 tensor_scalar
            for k in GEN_DVE:
                u = tpool_u.tile([PT, SH], f16, tag="tu", name=f"u{gt}_{h}_{k}")
                nc.vector.tensor_scalar(u[:], xs(k), scl2t[:, col(k)],
                                        bia2t[:, col(k)], Mult, Add)
                v = tpool_v.tile([PT, SH], f16, tag="tv", name=f"v{gt}_{h}_{k}")
                if k in PE_LANE:
                    nc.vector.tensor_scalar(v[:], u[:], 0.0, None, Max)
                else:
                    nc.vector.tensor_scalar(v[:], u[:], 0.0, sgnt[:, col(k)],
                                            Max, Mult)
                terms[k] = v

            # PSUM: 8 c-diag weighted term accumulations
            ps = zps.tile([PT, SH], f32, tag="zps", name=f"zps{gt}_{h}")
            for j, k in enumerate(PE_LANE):
                nc.tensor.matmul(ps[:], dg[:, j * PT:(j + 1) * PT],
                                 terms[k][:], start=(j == 0),
                                 stop=(j == len(PE_LANE) - 1))

            # DVE-lane: 3 adds + psum merge -> z fp16
            items = [terms[k] for k in DVE_LANE]
            d1 = accp.tile([PT, SH], f16, tag="acc", name=f"d1_{gt}_{h}")
            nc.vector.tensor_tensor(d1[:], items[0][:], items[1][:], Add)
            d2 = accp.tile([PT, SH], f16, tag="acc", name=f"d2_{gt}_{h}")
            nc.vector.tensor_tensor(d2[:], items[2][:], items[3][:], Add)
            nc.vector.tensor_tensor(d1[:], d1[:], d2[:], Add)
            z = zpool.tile([PT, SH], f16, tag="z", name=f"z{gt}_{h}")
            nc.vector.tensor_tensor(z[:], ps[:], d1[:], Add)
            z_half[h][gt] = z

        # ---------------- Phase B: one n-tile, one s-half --------------------
        partial_cr = [[p[:].rearrange("(g p) s -> p g s", p=PN)
                       for p in row] for row in partial_c]

        def issue_rs(j, h):
            nc.gpsimd.collective_compute(
                "ReduceScatter", Add,
                replica_groups=[list(range(NCORES))],
                ins=[partial_c[j][h].opt()], outs=[rs_c[j][h].opt()],
            )

        def phase_b(nt, h):
            if h == 0 and nt in w_tiles:
                w = w_tiles.pop(nt)
            else:
                w = load_w0(nt)
            pp = mmps.tile([PN, SH], f32, tag="mm", name=f"mm{nt}_{h}")
            for gt in range(NGT):
                nc.tensor.matmul(pp[:], w[:, gt * PN:(gt + 1) * PN],
                                 z_half[h][gt][:],
                                 start=(gt == 0), stop=(gt == NGT - 1))
            o = opool.tile([PN, SH], f16, tag="o1", name=f"o{nt}_{h}")
            nc.scalar.copy(o[:], pp[:])
            j, ng2 = divmod(nt, NNT // NCH)
            nc.sync.dma_start(partial_cr[j][h][:, ng2, :], o[:])
            if ng2 == NNT // NCH - 1:
                issue_rs(j, h)

        # ---------------- emission schedule ----------------
        # h0 sweep; keep x h0 loads 3 tiles ahead, dg loads 4 ahead
        for gt in range(NGT):
            if gt + 3 < NGT:
                x_h0[gt + 3] = load_x(gt + 3, 0, xpool, nc.gpsimd)
            if gt + 4 < NGT:
                dgn = dgpool.tile([PT, NDIAG * PT], f16, tag="dg",
                                  name=f"dg{gt + 4}")
                nc.gpsimd.dma_start(dgn[:], d["pediag"][gt + 4])
                dg_tiles[gt + 4] = dgn
            phase_a(gt, 0)
            if gt >= NGT - 4:       # prefetch first h1 tiles near sweep end
                x_h1[gt - (NGT - 4)] = load_x(gt - (NGT - 4), 1, x1pool,
                                              nc.gpsimd)

        # h1 sweep interleaved with B(h0) emission (keeps every engine's
        # queue in rough execution order)
        nb = 0
        for gt in range(NGT):
            if gt + 4 < NGT:
                x_h1[gt + 4] = load_x(gt + 4, 1, x1pool, nc.gpsimd)
            phase_a(gt, 1)
            while nb * NGT < (gt + 1) * NNT:
                phase_b(nb, 0)
                nb += 1

        for nt in range(NNT):
            phase_b(nt, 1)

        # ------- Phase D: distributed 2000->200, AllReduce, tail -------
        p2sb = []
        for j in range(NCH):
            y1 = tpool.tile([PN, S], f16, tag=f"y1_{j}", name=f"y1_{j}")
            for h in range(2):
                nc.sync.dma_start(y1[:, h * SH:(h + 1) * SH], rs_c[j][h][:])
            z1 = tpool.tile([PN, S], f16, tag=f"z1_{j}", name=f"z1_{j}")
            nc.scalar.activation(z1[:], y1[:], Relu,
                                 bias=cb0t[:, j:j + 1], scale=1.0)
            p2j = tpool.tile([100, 2 * S], f16, tag=f"p2sb{j}",
                             name=f"p2sb_{j}")
            p2sb.append(p2j)
            for mc in range(2):
                for sh in range(2):
                    ps = mmps.tile([100, SH], f32, tag="mm",
                                   name=f"ps2_{j}{mc}{sh}")
                    nc.tensor.matmul(
                        ps[:], w1t[:, j * N2 + mc * 100:j * N2 + (mc + 1) * 100],
                        z1[:, sh * SH:(sh + 1) * SH], start=True, stop=True)
                    nc.scalar.copy(p2j[:, mc * S + sh * SH:mc * S + (sh + 1) * SH],
                                   ps[:])
        # sum the two chunk partials, single fp16 AllReduce
        nc.vector.tensor_tensor(p2sb[0][:], p2sb[0][:], p2sb[1][:], Add)
        for mc in range(2):
            nc.sync.dma_start(partial2[mc * 100:(mc + 1) * 100, :],
                              p2sb[0][:, mc * S:(mc + 1) * S])
        nc.gpsimd.collective_compute(
            "AllReduce", Add,
            replica_groups=[list(range(NCORES))],
            ins=[partial2.opt()], outs=[summed2.opt()],
        )
        z2all = tpool.tile([100, 2 * S], f16, tag="z2")
        z2_tiles = [z2all[:, 0:S], z2all[:, S:2 * S]]
        for mc in range(2):
            y2a = tpool.tile([100, S], f16, tag=f"y2a_{mc}", name=f"y2a_{mc}")
            nc.sync.dma_start(y2a[:], summed2[mc * 100:(mc + 1) * 100, :])
            nc.scalar.activation(z2_tiles[mc][:], y2a[:], Relu,
                                 bias=cb1t[:, mc:mc + 1], scale=1.0)
        z3 = tpool.tile([N3, S], f16, tag="z3")
        for sh in range(2):
            ps = mmps.tile([N3, SH], f32, tag="mm")
            for mc in range(2):
                nc.tensor.matmul(ps[:], cw2tt[:, mc * N3:(mc + 1) * N3],
                                 z2_tiles[mc][:, sh * SH:(sh + 1) * SH],
                                 start=(mc == 0), stop=(mc == 1))
            nc.scalar.activation(z3[:, sh * SH:(sh + 1) * SH], ps[:], Relu,
                                 bias=cb2t[:], scale=1.0)
        outt = tpool.tile([1, S], f32, tag="outt")
        for sh in range(2):
            ps = mmps.tile([1, SH], f32, tag="mm")
            nc.tensor.matmul(ps[:], cwftt[:],
                             z3[:, sh * SH:(sh + 1) * SH],
                             start=True, stop=True)
            nc.scalar.activation(outt[:, sh * SH:(sh + 1) * SH], ps[:], Ident,
                                 bias=cbft[:], scale=1.0)
        nc.sync.dma_start(out_d[:], outt[:])

    nc.compile()
    return nc


def _shard_inputs(x, W1, b1, W2, b2, Wc, bc,
                  CW0, Cb0, CW1, Cb1, CW2, Cb2, CWf, Cbf):
    f = lambda a: np.ascontiguousarray(a, dtype=np.float32)
    h16 = lambda a: np.ascontiguousarray(a).astype(np.float16)
    CW1T = np.ascontiguousarray(CW1.T)
    shared = {
        "cb1": f(Cb1.reshape(2, 100).T),
        "cw2t": h16(CW2.T),
        "cb2": f(Cb2.reshape(N3, 1)),
        "cwft": h16(CWf.T),
        "cbf": f(Cbf.reshape(1, 1)),
    }

    # c_k = W2 * Wc  (T,G,H); relu-form: c*relu(s x + b) = sgn*relu(|c|s x + |c|b)
    # PE_LANE terms keep raw (s, b) -- the signed c-diagonal applies the weight.
    c = W2.astype(np.float64) * Wc.T.astype(np.float64)[:, :, None]
    ca = np.abs(c)
    scl2 = ca * W1.astype(np.float64)                 # (T,G,H)
    bia2 = ca * b1.astype(np.float64)
    sgn = np.where(c >= 0, 1.0, -1.0)
    for k in PE_LANE:
        t, hh = divmod(k, H)
        scl2[t, :, hh] = W1[t, :, hh]
        bia2[t, :, hh] = b1[t, :, hh]
    # (DVE_LANE keeps |c| folded into scl2/bia2 with sgn applied post-relu)
    # fold: sum_t b2*Wc + bc  -> Cb0
    fold = ((b2.astype(np.float64) * Wc.T.astype(np.float64)).sum(axis=0)
            + bc.astype(np.float64))                  # (G,)
    Cb0_adj = Cb0.astype(np.float64) + CW0.astype(np.float64) @ fold

    def per_gene(arr):   # (T, GL, H) -> (GL, NK), k = t*H+h
        return arr.transpose(1, 0, 2).reshape(GL, NK)

    in_maps = []
    for cix in range(NCORES):
        gs = slice(cix * GL, (cix + 1) * GL)
        scl2c = per_gene(scl2[:, gs, :])
        bia2c = per_gene(bia2[:, gs, :])
        sgnc = per_gene(sgn[:, gs, :])

        # pediag: (NGT, PT, NDIAG*PT): signed c diagonals for PE_LANE terms
        cc = per_gene(c[:, gs, :])
        pediag = np.zeros((NGT, PT, NDIAG * PT), dtype=np.float64)
        idx = np.arange(PT)
        for gt in range(NGT):
            rows = slice(gt * PT, (gt + 1) * PT)
            for j, k in enumerate(PE_LANE):
                pediag[gt, idx, j * PT + idx] = cc[rows, k]

        def col_layout(a):   # (GL, NK) -> (PT, NGT*NK)
            return f(a.reshape(NGT, PT, NK).transpose(1, 0, 2)
                     .reshape(PT, NGT * NK))

        half = N1 // 2
        in_maps.append({
            "xT": np.ascontiguousarray(
                x[:, :, gs].transpose(2, 0, 1).reshape(NGT, PT, T * S)
            ).astype(np.float16),
            "scl2": col_layout(scl2c),
            "bia2": col_layout(bia2c),
            "sgn": col_layout(sgnc),
            "pediag": pediag.astype(np.float16),
            "cw0t": np.ascontiguousarray(
                CW0[:, gs].reshape(NNT, PN, NGT, PT)
                .transpose(0, 3, 2, 1).reshape(NNT, PT, NGT * PN)
            ).astype(np.float16),
            "cb0": f(np.stack([Cb0_adj[cix * PN:(cix + 1) * PN],
                               Cb0_adj[half + cix * PN:half + (cix + 1) * PN]],
                              axis=1)),
            "cw1t": h16(np.concatenate(
                [CW1T[cix * PN:(cix + 1) * PN, :],
                 CW1T[half + cix * PN:half + (cix + 1) * PN, :]], axis=1)),
            **shared,
        })
    return in_maps


def _install_profile_shim():
    """Register the NTFF profiling hook that this container's antenv lacks."""
    import types
    try:
        import antenv.axon_hooks  # noqa: F401
        return True
    except ImportError:
        pass
    try:
        import antenv
        from trn_agent_boot.trn_boot import _ntff_profile_via_ctypes
        hook = _ntff_profile_via_ctypes("/opt/axon/libaxon_pjrt.so")
        mod = types.ModuleType("antenv.axon_hooks")
        mod.get_axon_ntff_profile_hook = lambda: hook
        mod.set_axon_ntff_profile_hook = lambda h: None
        sys.modules["antenv.axon_hooks"] = mod
        antenv.axon_hooks = mod
        return hook is not None
    except Exception:
        return False


def kernel(**inputs):
    inputs = {k: np.asarray(v) for k, v in inputs.items()}
    in_maps = _shard_inputs(**inputs)
    if "nc" not in _CACHE:
        _CACHE["nc"] = _build_program()
    nc = _CACHE["nc"]
    trace = bool(os.environ.get("KERNEL_PROFILE")) and _install_profile_shim()
    res = run_bass_kernel_spmd(nc, in_maps, core_ids=list(range(NCORES)),
                               trace=trace)
    LAST_RUN["exec_time_ns"] = res.exec_time_ns
    LAST_RUN["mean_exec_time_ns"] = res.mean_exec_time_ns
    if res.instructions_and_trace is not None:
        LAST_RUN["trace_path"] = res.instructions_and_trace[1]
    return res.results[0]["out"].reshape(1, S, 1)


if __name__ == "__main__":
    rng = np.random.default_rng(0)
    ins = {
        "x": rng.standard_normal((T, S, G), dtype=np.float32),
        "W1": rng.standard_normal((T, G, H), dtype=np.float32) * 0.5,
        "b1": rng.standard_normal((T, G, H), dtype=np.float32) * 0.1,
        "W2": rng.standard_normal((T, G, H), dtype=np.float32) * 0.5,
        "b2": rng.standard_normal((T, G), dtype=np.float32) * 0.1,
        "Wc": rng.standard_normal((G, T), dtype=np.float32) * 0.5,
        "bc": rng.standard_normal((G,), dtype=np.float32) * 0.1,
        "CW0": rng.standard_normal((N1, G), dtype=np.float32) * 0.007,
        "Cb0": rng.standard_normal((N1,), dtype=np.float32) * 0.007,
        "CW1": rng.standard_normal((N2, N1), dtype=np.float32) * 0.02,
        "Cb1": rng.standard_normal((N2,), dtype=np.float32) * 0.02,
        "CW2": rng.standard_normal((N3, N2), dtype=np.float32) * 0.07,
        "Cb2": rng.standard_normal((N3,), dtype=np.float32) * 0.07,
        "CWf": rng.standard_normal((1, N3), dtype=np.float32) * 0.2,
        "Cbf": rng.standard_normal((1,), dtype=np.float32) * 0.2,
    }
    out = kernel(**ins)
    xx = ins["x"]
    h = np.maximum(xx[..., None] * ins["W1"][:, None] + ins["b1"][:, None], 0.0)
    y = np.einsum("tsgh,tgh->tsg", h, ins["W2"]) + ins["b2"][:, None, :]
    zz = np.einsum("tsg,gt->sg", y, ins["Wc"]) + ins["bc"]
    for Wl, bl in ((ins["CW0"], ins["Cb0"]), (ins["CW1"], ins["Cb1"]),
                   (ins["CW2"], ins["Cb2"])):
        zz = np.maximum(zz @ Wl.T + bl, 0.0)
    ref = (zz @ ins["CWf"].T + ins["Cbf"])[None]
    err = np.abs(out - ref).max() / (np.abs(ref).max() + 1e-12)
    print("self-test rel err:", err)
    print("exec_time_ns:", LAST_RUN.get("exec_time_ns"))

